# revision 28
# baseline (speedup 1.0000x reference)
"""Trainium2 Bass kernel: Gaussian-RBF basis expansion + batched matmul.

Computes, for B=32 batches, N=65536 positions, DEG=32 basis functions,
D=8 output dims:
    basis[b,n,g] = exp(-(x[b,n] - c_g)^2 / (2*0.04))
    result[b,n,d] = sum_g basis[b,n,g] * weights[b,d,g]
and returns (result, zeros_like(result)).

Strategy (8 NeuronCores, shard N; raw Bass, hand-rolled software
pipeline with explicit semaphores):
  * Factor the Gaussian: exp(-12.5(x-c)^2) = A(x) * exp(25c*x - 12.5c^2)
    with A(x) = exp(-12.5 x^2).  The second factor is ONE ScalarE
    activation per element (per-partition scale/bias) — the minimal exp
    work: 32 exps per (b,n) at 1 elem/cycle/lane.
  * K-pack the matmul: K = 128 = 8 degree-rows x 16 batches, block
    diagonal weights; accumulate 4 chunks of 8 degrees into PSUM.
    M = 128 = 8 dims x 16 batches (d-major), moving N = positions.
    E streams as float32r (single-pass full-rate fp32, ~tf32 rounding).
  * The 16->128 partition broadcasts of x (K layout) and A (M layout)
    run on the TensorEngine as K=32 indicator matmuls (per-group
    selector over the q-major packed [128, 2048] layout, 32-aligned
    bases) with bf16 hi+lo two-pass accumulation (exact to ~2^-17) —
    16-partition DMA broadcasts ran at ~1/3 fabric bandwidth and
    dominated v1 (200 us).
  * A(x) computed once on the packed layout (full 128 lanes) and split
    hi/lo on VectorE; no re-layout needed (packed layout slices are the
    broadcast matmul inputs directly).
  * Device output layout [group, (d*16+b), n] so every DMA is dense;
    the final [B, N, D] transpose happens on host.
"""

import numpy as np
from contextlib import ExitStack

import concourse.bass as bass
from concourse import mybir
from concourse.bass_utils import run_bass_kernel_spmd

# Problem constants (hardcoded per harness contract)
B, D, N, DEG = 32, 8, 65536, 32
SCALE = 0.04
INV2S = 1.0 / (2.0 * SCALE)  # 12.5
NCORES = 8
NSH = N // NCORES  # 8192 positions per core

# Kernel layout constants
T2 = 2048          # positions per pipeline iteration
SUB = 512          # matmul moving-free-dim (one fp32 PSUM bank)
NQ = NSH // T2     # 4 position blocks per core
GB = 16            # batches per group (K = 8*16 = 128)
NG = B // GB       # 2 batch groups
PG = 8             # degrees per matmul chunk
NCHUNK = DEG // PG # 4 matmul accumulation chunks
NIT = NG * NQ      # 8 pipeline iterations
NSUB = T2 // SUB   # 4 matmul sub-tiles per iteration

F32R = True        # stream E-matmul operands as float32r

FP = mybir.dt.float32
BF = mybir.dt.bfloat16

_centers = np.linspace(0.0, 1.01, DEG).astype(np.float64)


def _build():
    nc = bass.Bass(
        "TRN2", target_bir_lowering=False, debug=False, num_devices=NCORES
    )
    MMDT = mybir.dt.float32r if F32R else FP
    xpk_d = nc.dram_tensor("xpk", [128, T2], FP, kind="ExternalInput")
    xhi_d = nc.dram_tensor("xhi", [128, T2], BF, kind="ExternalInput")
    xlo_d = nc.dram_tensor("xlo", [128, T2], BF, kind="ExternalInput")
    lhsT_d = nc.dram_tensor(
        "lhsTw", [128, NG, NCHUNK, 128], MMDT, kind="ExternalInput"
    )
    lbc_d = nc.dram_tensor("lbc", [128, NG, 128], BF, kind="ExternalInput")
    scbi_d = nc.dram_tensor("scbi", [128, 2 * NCHUNK], FP, kind="ExternalInput")
    out_d = nc.dram_tensor("out", [NG, 128, NSH], FP, kind="ExternalOutput")

    EXP = mybir.ActivationFunctionType.Exp

    with ExitStack() as ctx:
        en = ctx.enter_context
        # --- SBUF tensors -------------------------------------------------
        xpk = en(nc.sbuf_tensor("xpk_sb", [128, T2], FP)).ap()
        xhi = en(nc.sbuf_tensor("xhi_sb", [128, T2], BF)).ap()
        xlo = en(nc.sbuf_tensor("xlo_sb", [128, T2], BF)).ap()
        lhsT = en(nc.sbuf_tensor("lhsT_sb", [128, NG, NCHUNK, 128], MMDT)).ap()
        lbc = en(nc.sbuf_tensor("lbc_sb", [128, NG, 128], BF)).ap()
        scbi = en(nc.sbuf_tensor("scbi_sb", [128, 2 * NCHUNK], FP)).ap()
        # A(x) path (one-time); sq/res reuse nothing else live at that point
        sqpk = en(nc.sbuf_tensor("sqpk_sb", [128, T2], FP)).ap()
        apk = en(nc.sbuf_tensor("apk_sb", [128, T2], FP)).ap()
        ahipk = en(nc.sbuf_tensor("ahipk_sb", [128, T2], BF)).ap()
        alopk = en(nc.sbuf_tensor("alopk_sb", [128, T2], BF)).ap()
        # pipeline tiles
        xbs = [en(nc.sbuf_tensor(f"xbs{i}", [128, T2], FP)).ap() for i in range(2)]
        a8s = [en(nc.sbuf_tensor(f"a8s{i}", [128, T2], FP)).ap() for i in range(2)]
        e = [
            [en(nc.sbuf_tensor(f"e{i}_{j}", [128, T2], MMDT)).ap()
             for j in range(NCHUNK)]
            for i in range(2)
        ]
        osb2 = [en(nc.sbuf_tensor(f"osb2_{i}", [128, T2], FP)).ap() for i in range(2)]
        # --- PSUM: 2 + 2 + 4 banks ---------------------------------------
        xbp = [en(nc.psum_tensor(f"xbp{i}", [128, SUB], FP)).ap() for i in range(2)]
        a8p = [en(nc.psum_tensor(f"a8p{i}", [128, SUB], FP)).ap() for i in range(2)]
        mmp = [en(nc.psum_tensor(f"mmp{s}", [128, SUB], FP)).ap() for s in range(NSUB)]
        # --- semaphores ---------------------------------------------------
        s_in = en(nc.semaphore("s_in"))
        s_sq = en(nc.semaphore("s_sq"))      # DVE sqpk done
        s_apk = en(nc.semaphore("s_apk"))    # ACT apk done
        s_asp = en(nc.semaphore("s_asp"))    # DVE A hi/lo split done
        s_xbp = en(nc.semaphore("s_xbp"))    # PE xb-bcast sub done (+1)
        s_a8p = en(nc.semaphore("s_a8p"))    # PE a8-bcast sub done (+1)
        s_xbc = en(nc.semaphore("s_xbc"))    # DVE xb-psum->sbuf copy done (+1)
        s_a8c = en(nc.semaphore("s_a8c"))    # DVE a8-psum->sbuf copy done (+1)
        s_e = en(nc.semaphore("s_e"))        # ACT chunk-exp done (+1)
        s_mm = en(nc.semaphore("s_mm"))      # PE E-mm sub complete (+1 at j=3)
        s_dvet = en(nc.semaphore("s_dvet"))  # DVE A-apply TT done (+1)
        s_out = en(nc.semaphore("s_out"))    # out DMA done (+16)

        NIN = 6  # input DMA count

        with nc.Block() as block:

            @block.sync
            def _(sync):
                sync.dma_start(out=xpk, in_=xpk_d.ap()).then_inc(s_in, 16)
                sync.dma_start(out=lhsT, in_=lhsT_d.ap()).then_inc(s_in, 16)
                sync.dma_start(out=scbi, in_=scbi_d.ap()).then_inc(s_in, 16)
                sync.dma_start(out=lbc, in_=lbc_d.ap()).then_inc(s_in, 16)
                sync.dma_start(out=xhi, in_=xhi_d.ap()).then_inc(s_in, 16)
                sync.dma_start(out=xlo, in_=xlo_d.ap()).then_inc(s_in, 16)
                # output DMAs
                for it in range(NIT):
                    g, q = divmod(it, NQ)
                    sync.wait_ge(s_dvet, NSUB * (it + 1))
                    sync.dma_start(
                        out=out_d.ap()[g, :, T2 * q : T2 * (q + 1)],
                        in_=osb2[it % 2],
                    ).then_inc(s_out, 16)

            @block.vector
            def _(vector):
                # one-time A path
                vector.wait_ge(s_in, 16)
                vector.tensor_mul(sqpk, xpk, xpk).then_inc(s_sq, 1)
                vector.wait_ge(s_apk, 1)
                vector.tensor_copy(ahipk, apk)             # fp32 -> bf16 hi
                vector.tensor_sub(sqpk, apk, ahipk)        # residual (reuse sqpk)
                vector.tensor_copy(alopk, sqpk).then_inc(s_asp, 1)
                # pipeline: the A-apply TTs lag the copies by one iteration,
                # mirroring the PE's lagged E-matmuls (avoids a cross-engine
                # wait cycle between next-iter broadcasts and this-iter TTs)
                def tts(jt):
                    bj = jt % 2
                    for s in range(NSUB):
                        idx = NSUB * jt + s
                        sl = slice(s * SUB, (s + 1) * SUB)
                        vector.wait_ge(s_mm, idx + 1)
                        if jt >= 2 and s == 0:
                            vector.wait_ge(s_out, 16 * (jt - 1))  # osb2 WAR
                        vector.tensor_mul(
                            osb2[bj][:, sl], mmp[s], a8s[bj][:, sl]
                        ).then_inc(s_dvet, 1)

                for it in range(NIT):
                    bi_ = it % 2
                    for s in range(NSUB):
                        idx = NSUB * it + s
                        sl = slice(s * SUB, (s + 1) * SUB)
                        vector.wait_ge(s_xbp, idx + 1)
                        if it >= 2 and s == 0:
                            vector.wait_ge(s_e, NCHUNK * (it - 1))  # xbs WAR
                        vector.tensor_copy(xbs[bi_][:, sl], xbp[s % 2]).then_inc(
                            s_xbc, 1
                        )
                    for s in range(NSUB):
                        idx = NSUB * it + s
                        sl = slice(s * SUB, (s + 1) * SUB)
                        vector.wait_ge(s_a8p, idx + 1)
                        vector.tensor_copy(a8s[bi_][:, sl], a8p[s % 2]).then_inc(
                            s_a8c, 1
                        )
                    if it >= 1:
                        tts(it - 1)
                tts(NIT - 1)

            @block.scalar
            def _(scalar):
                scalar.wait_ge(s_in, 16 * NIN)
                scalar.wait_ge(s_sq, 1)
                scalar.activation(apk, sqpk, EXP, scale=-INV2S).then_inc(s_apk, 1)
                for it in range(NIT):
                    bi_ = it % 2
                    scalar.wait_ge(s_xbc, NSUB * (it + 1))
                    if it >= 2:
                        scalar.wait_ge(s_mm, NSUB * (it - 1))  # e WAR
                    for j in range(NCHUNK):
                        scalar.activation(
                            e[bi_][j], xbs[bi_], EXP,
                            scale=scbi[:, j : j + 1],
                            bias=scbi[:, NCHUNK + j : NCHUNK + j + 1],
                        ).then_inc(s_e, 1)

            @block.tensor
            def _(tensor):
                def bcasts(it):
                    g, q = divmod(it, NQ)
                    rows32 = slice(32 * q, 32 * (q + 1))
                    # x broadcast: K=32 indicator matmul, hi+lo passes
                    for s in range(NSUB):
                        idx = NSUB * it + s
                        sl = slice(s * SUB, (s + 1) * SUB)
                        if idx >= 2:
                            tensor.wait_ge(s_xbc, idx - 1)  # xbp WAR
                        tensor.matmul(
                            xbp[s % 2], lbc[rows32, g, :], xhi[rows32, sl],
                            start=True, stop=False, skip_group_check=True,
                            tile_position=(32 * q, 0),
                        )
                        tensor.matmul(
                            xbp[s % 2], lbc[rows32, g, :], xlo[rows32, sl],
                            start=False, stop=True, skip_group_check=True,
                            tile_position=(32 * q, 0),
                        ).then_inc(s_xbp, 1)
                    # A broadcast: same indicator, M layout
                    if it == 0:
                        tensor.wait_ge(s_asp, 1)
                    for s in range(NSUB):
                        idx = NSUB * it + s
                        sl = slice(s * SUB, (s + 1) * SUB)
                        if idx >= 2:
                            tensor.wait_ge(s_a8c, idx - 1)  # a8p WAR
                        tensor.matmul(
                            a8p[s % 2], lbc[rows32, g, :], ahipk[rows32, sl],
                            start=True, stop=False, skip_group_check=True,
                            tile_position=(32 * q, 0),
                        )
                        tensor.matmul(
                            a8p[s % 2], lbc[rows32, g, :], alopk[rows32, sl],
                            start=False, stop=True, skip_group_check=True,
                            tile_position=(32 * q, 0),
                        ).then_inc(s_a8p, 1)

                def emms(it):
                    # E matmuls, j-outer. Coarse waits up front (the lag means
                    # they are satisfied long before) keep the PE queue dense
                    # so LDWEIGHTS pull-ahead can overlap the matmul stream.
                    bi_ = it % 2
                    g = it // NQ
                    tensor.wait_ge(s_e, NCHUNK * (it + 1))
                    if it >= 1:
                        tensor.wait_ge(s_dvet, NSUB * it)
                    for j in range(NCHUNK):
                        for s in range(NSUB):
                            sl = slice(s * SUB, (s + 1) * SUB)
                            mm = tensor.matmul(
                                mmp[s],
                                lhsT[:, g, j, :],
                                e[bi_][j][:, sl],
                                start=(j == 0),
                                stop=(j == NCHUNK - 1),
                                skip_group_check=True,
                            )
                            if j == NCHUNK - 1:
                                mm.then_inc(s_mm, 1)

                tensor.wait_ge(s_in, 16 * NIN)
                # E-matmuls lag the broadcasts by one iteration so the PE has
                # broadcast work to run while ScalarE produces the E tiles.
                for it in range(NIT):
                    bcasts(it)
                    if it >= 1:
                        emms(it - 1)
                emms(NIT - 1)
    return nc


def _split_hi_lo(x):
    """fp32 -> (hi, lo) fp32 pair where hi is bf16-representable and
    x == hi + lo exactly in fp32; bf16(lo) then loses only ~2^-17."""
    xb = np.ascontiguousarray(x.astype(np.float32))
    hi_bits = (xb.view(np.uint32) + 0x8000) & 0xFFFF0000
    hi = hi_bits.view(np.float32)
    lo = xb - hi
    return hi, lo


def _host_inputs(weights, positions):
    """Builds the per-core in_maps (host-side packing only, no math on data)."""
    import ml_dtypes

    w = np.ascontiguousarray(np.asarray(weights, dtype=np.float32))
    x = np.ascontiguousarray(np.asarray(positions, dtype=np.float32))

    # block-diagonal stationary operand
    # lhsT[k=(gg*16+b'), g, j, m=(d*16+b)] = delta(b,b') * w[16g+b, d, 8j+gg]
    w4 = w.reshape(NG, GB, D, NCHUNK, PG)  # [g, b, d, j, gg]
    eye = np.eye(GB, dtype=np.float32)
    lhsT = np.einsum("gbdjh,cb->hcgjdb", w4, eye)  # [gg, b', g, j, d, b]
    lhsT = np.ascontiguousarray(lhsT.reshape(128, NG, NCHUNK, 128))

    # broadcast selector for K=32 packed rows: row b2 of a 32-row block holds
    # batch b2 (0..31); group-g output column m wants batch 16g + m%16
    b2 = np.arange(32)[:, None]
    lbc = np.zeros((128, NG, 128), dtype=np.float32)
    for g in range(NG):
        sel = (b2 == (GB * g + np.arange(128)[None, :] % GB)).astype(np.float32)
        lbc[:, g, :] = np.tile(sel, (4, 1))
    lbc = lbc.astype(ml_dtypes.bfloat16)

    # per-partition activation scale/bias, partition p -> degree g = 8j + p//16
    gidx = np.arange(128) // GB
    scbi = np.zeros((128, 2 * NCHUNK), dtype=np.float32)
    for j in range(NCHUNK):
        c = _centers[PG * j + gidx]
        scbi[:, j] = (2.0 * INV2S) * c              # 25 c
        scbi[:, NCHUNK + j] = -INV2S * c * c        # -12.5 c^2

    in_maps = []
    for ci in range(NCORES):
        xs = x[:, ci * NSH : (ci + 1) * NSH]  # [32, NSH]
        # q-major packing: row q*32+b holds xs[b, 2048q:2048(q+1)]
        xpk = np.ascontiguousarray(
            xs.reshape(B, NQ, T2).transpose(1, 0, 2).reshape(128, T2)
        )
        hi, lo = _split_hi_lo(xpk)
        in_maps.append(
            {
                "xpk": xpk,
                "xhi": np.ascontiguousarray(hi.astype(ml_dtypes.bfloat16)),
                "xlo": np.ascontiguousarray(lo.astype(ml_dtypes.bfloat16)),
                "lhsTw": lhsT,
                "lbc": lbc,
                "scbi": scbi,
            }
        )
    return in_maps


def _gather(results):
    """[NG, 128, NSH] per core, rows m=d*16+b  ->  full [B, N, D]."""
    outs = []
    for r in results:
        o = r["out"].reshape(NG, D, GB, NSH)  # [g, d, b, n]
        outs.append(o.transpose(0, 2, 3, 1).reshape(B, NSH, D))  # [b, n, d]
    full = np.concatenate(outs, axis=1)  # [B, N, D]
    return np.ascontiguousarray(full)


_NC_CACHE = {}


def run(inputs, trace=False, **trace_kwargs):
    """Builds (cached), runs on 8 cores, returns ((result, zeros), BassKernelResults)."""
    key = (F32R,)
    if key not in _NC_CACHE:
        _NC_CACHE[key] = _build()
    nc = _NC_CACHE[key]
    in_maps = _host_inputs(inputs["weights"], inputs["positions"])
    br = run_bass_kernel_spmd(
        nc, in_maps, list(range(NCORES)), trace=trace, **trace_kwargs
    )
    result = _gather(br.results)
    return (result, np.zeros_like(result)), br


def kernel(weights, weights_std, positions):
    out, _ = run(
        {"weights": weights, "weights_std": weights_std, "positions": positions}
    )
    return out


# revision 31
# speedup vs baseline: 1.0733x; 1.0733x over previous
"""Trainium2 Bass kernel: Gaussian-RBF basis expansion + batched matmul.

Computes, for B=32 batches, N=65536 positions, DEG=32 basis functions,
D=8 output dims:
    basis[b,n,g] = exp(-(x[b,n] - c_g)^2 / (2*0.04))
    result[b,n,d] = sum_g basis[b,n,g] * weights[b,d,g]
and returns (result, zeros_like(result)).

Strategy (8 NeuronCores, shard N; raw Bass, hand-rolled software
pipeline with explicit semaphores):
  * Factor the Gaussian: exp(-12.5(x-c)^2) = A(x) * exp(25c*x - 12.5c^2)
    with A(x) = exp(-12.5 x^2).  The second factor is ONE ScalarE
    activation per element (per-partition scale/bias) — the minimal exp
    work: 32 exps per (b,n) at 1 elem/cycle/lane.
  * K-pack the matmul: K = 128 = 8 degree-rows x 16 batches, block
    diagonal weights; accumulate 4 chunks of 8 degrees into PSUM.
    M = 128 = 8 dims x 16 batches (d-major), moving N = positions.
    E streams as float32r (single-pass full-rate fp32, ~tf32 rounding).
  * The 16->128 partition broadcasts of x (K layout) and A (M layout)
    run on the TensorEngine as K=32 indicator matmuls (per-group
    selector over the q-major packed [128, 2048] layout, 32-aligned
    bases) with bf16 hi+lo two-pass accumulation (exact to ~2^-17) —
    16-partition DMA broadcasts ran at ~1/3 fabric bandwidth and
    dominated v1 (200 us).
  * A(x) computed once on the packed layout (full 128 lanes) and split
    hi/lo on VectorE; no re-layout needed (packed layout slices are the
    broadcast matmul inputs directly).
  * Device output layout [group, (d*16+b), n] so every DMA is dense;
    the final [B, N, D] transpose happens on host.
"""

import numpy as np
from contextlib import ExitStack

import concourse.bass as bass
from concourse import mybir
from concourse.bass_utils import run_bass_kernel_spmd

# Problem constants (hardcoded per harness contract)
B, D, N, DEG = 32, 8, 65536, 32
SCALE = 0.04
INV2S = 1.0 / (2.0 * SCALE)  # 12.5
NCORES = 8
NSH = N // NCORES  # 8192 positions per core

# Kernel layout constants
T2 = 2048          # positions per pipeline iteration
SUB = 512          # matmul moving-free-dim (one fp32 PSUM bank)
NQ = NSH // T2     # 4 position blocks per core
GB = 16            # batches per group (K = 8*16 = 128)
NG = B // GB       # 2 batch groups
PG = 8             # degrees per matmul chunk
NCHUNK = DEG // PG # 4 matmul accumulation chunks
NIT = NG * NQ      # 8 pipeline iterations
NSUB = T2 // SUB   # 4 matmul sub-tiles per iteration

F32R = True        # stream E-matmul operands as float32r

FP = mybir.dt.float32
BF = mybir.dt.bfloat16

_centers = np.linspace(0.0, 1.01, DEG).astype(np.float64)


def _build():
    nc = bass.Bass(
        "TRN2", target_bir_lowering=False, debug=False, num_devices=NCORES
    )
    MMDT = mybir.dt.float32r if F32R else FP
    xpk_d = nc.dram_tensor("xpk", [128, T2], FP, kind="ExternalInput")
    xhl0_d = nc.dram_tensor("xhl0", [128, T2], BF, kind="ExternalInput")
    xhl1_d = nc.dram_tensor("xhl1", [128, T2], BF, kind="ExternalInput")
    lhsT_d = nc.dram_tensor(
        "lhsTw", [128, NG, NCHUNK, 128], MMDT, kind="ExternalInput"
    )
    lbc64_d = nc.dram_tensor("lbc64", [128, NG, 128], BF, kind="ExternalInput")
    lbc16_d = nc.dram_tensor("lbc16", [128, NG, 128], mybir.dt.float16,
                             kind="ExternalInput")
    scbi_d = nc.dram_tensor("scbi", [128, 2 * NCHUNK], FP, kind="ExternalInput")
    out_d = nc.dram_tensor("out", [NG, 128, NSH], FP, kind="ExternalOutput")

    EXP = mybir.ActivationFunctionType.Exp

    with ExitStack() as ctx:
        en = ctx.enter_context
        # --- SBUF tensors -------------------------------------------------
        xpk = en(nc.sbuf_tensor("xpk_sb", [128, T2], FP)).ap()
        xhl = [en(nc.sbuf_tensor(f"xhl{t}_sb", [128, T2], BF)).ap()
               for t in range(2)]
        lhsT = en(nc.sbuf_tensor("lhsT_sb", [128, NG, NCHUNK, 128], MMDT)).ap()
        lbc64 = en(nc.sbuf_tensor("lbc64_sb", [128, NG, 128], BF)).ap()
        lbc16 = en(nc.sbuf_tensor("lbc16_sb", [128, NG, 128],
                                  mybir.dt.float16)).ap()
        scbi = en(nc.sbuf_tensor("scbi_sb", [128, 2 * NCHUNK], FP)).ap()
        # A(x) path (one-time)
        sqpk = en(nc.sbuf_tensor("sqpk_sb", [128, T2], FP)).ap()
        apk = en(nc.sbuf_tensor("apk_sb", [128, T2], FP)).ap()
        a16 = en(nc.sbuf_tensor("a16_sb", [128, T2], mybir.dt.float16)).ap()
        # pipeline tiles
        xbs = [en(nc.sbuf_tensor(f"xbs{i}", [128, T2], FP)).ap() for i in range(2)]
        a8s = [en(nc.sbuf_tensor(f"a8s{i}", [128, T2], FP)).ap() for i in range(2)]
        e = [
            [en(nc.sbuf_tensor(f"e{i}_{j}", [128, T2], MMDT)).ap()
             for j in range(NCHUNK)]
            for i in range(2)
        ]
        osb2 = [en(nc.sbuf_tensor(f"osb2_{i}", [128, T2], FP)).ap() for i in range(2)]
        # --- PSUM: 2 + 2 + 4 banks ---------------------------------------
        xbp = [en(nc.psum_tensor(f"xbp{i}", [128, SUB], FP)).ap() for i in range(2)]
        a8p = [en(nc.psum_tensor(f"a8p{i}", [128, SUB], FP)).ap() for i in range(2)]
        mmp = [en(nc.psum_tensor(f"mmp{s}", [128, SUB], FP)).ap() for s in range(NSUB)]
        # --- semaphores ---------------------------------------------------
        s_in = en(nc.semaphore("s_in"))    # bcast inputs (lbc64,lbc16,xhl0,xhl1)
        s_in2 = en(nc.semaphore("s_in2"))  # xpk, lhsTw, scbi
        s_sq = en(nc.semaphore("s_sq"))      # DVE sqpk done
        s_apk = en(nc.semaphore("s_apk"))    # ACT apk done
        s_asp = en(nc.semaphore("s_asp"))    # DVE A hi/lo split done
        s_xbp = en(nc.semaphore("s_xbp"))    # PE xb-bcast sub done (+1)
        s_a8p = en(nc.semaphore("s_a8p"))    # PE a8-bcast sub done (+1)
        s_xbc = en(nc.semaphore("s_xbc"))    # DVE xb-psum->sbuf copy done (+1)
        s_a8c = en(nc.semaphore("s_a8c"))    # DVE a8-psum->sbuf copy done (+1)
        s_e = en(nc.semaphore("s_e"))        # ACT chunk-exp done (+1)
        s_mm = en(nc.semaphore("s_mm"))      # PE E-mm sub complete (+1 at j=3)
        s_dvet = en(nc.semaphore("s_dvet"))  # DVE A-apply TT done (+1)
        s_out = en(nc.semaphore("s_out"))    # out DMA done (+16)

        NIN = 7  # input DMA count

        with nc.Block() as block:

            @block.sync
            def _(sync):
                sync.dma_start(out=lbc64, in_=lbc64_d.ap()).then_inc(s_in, 16)
                sync.dma_start(out=lbc16, in_=lbc16_d.ap()).then_inc(s_in, 16)
                sync.dma_start(out=xhl[0], in_=xhl0_d.ap()).then_inc(s_in, 16)
                sync.dma_start(out=xhl[1], in_=xhl1_d.ap()).then_inc(s_in, 16)
                sync.dma_start(out=xpk, in_=xpk_d.ap()).then_inc(s_in2, 16)
                sync.dma_start(out=lhsT, in_=lhsT_d.ap()).then_inc(s_in2, 16)
                sync.dma_start(out=scbi, in_=scbi_d.ap()).then_inc(s_in2, 16)
                # output DMAs
                for it in range(NIT):
                    g, q = divmod(it, NQ)
                    sync.wait_ge(s_dvet, NSUB * (it + 1))
                    sync.dma_start(
                        out=out_d.ap()[g, :, T2 * q : T2 * (q + 1)],
                        in_=osb2[it % 2],
                    ).then_inc(s_out, 16)

            @block.vector
            def _(vector):
                # one-time A path
                vector.wait_ge(s_in2, 48)
                vector.tensor_mul(sqpk, xpk, xpk).then_inc(s_sq, 1)
                vector.wait_ge(s_apk, 1)
                # A' = A * 2^14 in fp16 (2^-14 is folded into lhsTw); fp16's
                # 11-bit mantissa matches the f32r rounding already in play
                vector.tensor_scalar_mul(a16, apk, 16384.0).then_inc(s_asp, 1)
                # pipeline: the A-apply TTs lag the copies by one iteration,
                # mirroring the PE's lagged E-matmuls (avoids a cross-engine
                # wait cycle between next-iter broadcasts and this-iter TTs)
                def tts(jt):
                    bj = jt % 2
                    for s in range(NSUB):
                        idx = NSUB * jt + s
                        sl = slice(s * SUB, (s + 1) * SUB)
                        vector.wait_ge(s_mm, idx + 1)
                        if jt >= 2 and s == 0:
                            vector.wait_ge(s_out, 16 * (jt - 1))  # osb2 WAR
                        vector.tensor_mul(
                            osb2[bj][:, sl], mmp[s], a8s[bj][:, sl]
                        ).then_inc(s_dvet, 1)

                for it in range(NIT):
                    bi_ = it % 2
                    for s in range(NSUB):
                        idx = NSUB * it + s
                        sl = slice(s * SUB, (s + 1) * SUB)
                        vector.wait_ge(s_xbp, idx + 1)
                        if it >= 2 and s == 0:
                            vector.wait_ge(s_e, NCHUNK * (it - 1))  # xbs WAR
                        vector.tensor_copy(xbs[bi_][:, sl], xbp[s % 2]).then_inc(
                            s_xbc, 1
                        )
                    for s in range(NSUB):
                        idx = NSUB * it + s
                        sl = slice(s * SUB, (s + 1) * SUB)
                        vector.wait_ge(s_a8p, idx + 1)
                        vector.tensor_copy(a8s[bi_][:, sl], a8p[s % 2]).then_inc(
                            s_a8c, 1
                        )
                    if it >= 1:
                        tts(it - 1)
                tts(NIT - 1)

            @block.scalar
            def _(scalar):
                scalar.wait_ge(s_in2, 48)
                scalar.wait_ge(s_sq, 1)
                scalar.activation(apk, sqpk, EXP, scale=-INV2S).then_inc(s_apk, 1)
                for it in range(NIT):
                    bi_ = it % 2
                    scalar.wait_ge(s_xbc, NSUB * (it + 1))
                    if it >= 2:
                        scalar.wait_ge(s_mm, NSUB * (it - 1))  # e WAR
                    for j in range(NCHUNK):
                        scalar.activation(
                            e[bi_][j], xbs[bi_], EXP,
                            scale=scbi[:, j : j + 1],
                            bias=scbi[:, NCHUNK + j : NCHUNK + j + 1],
                        ).then_inc(s_e, 1)

            @block.tensor
            def _(tensor):
                def bcasts(it):
                    g, q = divmod(it, NQ)
                    t, qq = divmod(q, 2)
                    rows64 = slice(64 * qq, 64 * (qq + 1))
                    rows32 = slice(32 * q, 32 * (q + 1))
                    # x broadcast: K=64 indicator matmul, hi|lo stacked rows
                    for s in range(NSUB):
                        idx = NSUB * it + s
                        sl = slice(s * SUB, (s + 1) * SUB)
                        if idx >= 2:
                            tensor.wait_ge(s_xbc, idx - 1)  # xbp WAR
                        tensor.matmul(
                            xbp[s % 2], lbc64[rows64, g, :], xhl[t][rows64, sl],
                            start=True, stop=True, skip_group_check=True,
                            tile_position=(64 * qq, 0),
                        ).then_inc(s_xbp, 1)
                    # A broadcast: fp16 single pass (prescaled by 2^14)
                    if it == 0:
                        tensor.wait_ge(s_asp, 1)
                    for s in range(NSUB):
                        idx = NSUB * it + s
                        sl = slice(s * SUB, (s + 1) * SUB)
                        if idx >= 2:
                            tensor.wait_ge(s_a8c, idx - 1)  # a8p WAR
                        tensor.matmul(
                            a8p[s % 2], lbc16[rows32, g, :], a16[rows32, sl],
                            start=True, stop=True, skip_group_check=True,
                            tile_position=(32 * q, 0),
                        ).then_inc(s_a8p, 1)

                def emms(it):
                    # E matmuls, j-outer. Coarse waits up front (the lag means
                    # they are satisfied long before) keep the PE queue dense
                    # so LDWEIGHTS pull-ahead can overlap the matmul stream.
                    bi_ = it % 2
                    g = it // NQ
                    tensor.wait_ge(s_e, NCHUNK * (it + 1))
                    if it >= 1:
                        tensor.wait_ge(s_dvet, NSUB * it)
                    for j in range(NCHUNK):
                        for s in range(NSUB):
                            sl = slice(s * SUB, (s + 1) * SUB)
                            mm = tensor.matmul(
                                mmp[s],
                                lhsT[:, g, j, :],
                                e[bi_][j][:, sl],
                                start=(j == 0),
                                stop=(j == NCHUNK - 1),
                                skip_group_check=True,
                            )
                            if j == NCHUNK - 1:
                                mm.then_inc(s_mm, 1)

                tensor.wait_ge(s_in, 64)
                # E-matmuls lag the broadcasts by one iteration so the PE has
                # broadcast work to run while ScalarE produces the E tiles.
                for it in range(NIT):
                    bcasts(it)
                    if it == 0:
                        tensor.wait_ge(s_in2, 48)  # lhsTw landed
                    if it >= 1:
                        emms(it - 1)
                emms(NIT - 1)
    return nc


def _split_hi_lo(x):
    """fp32 -> (hi, lo) fp32 pair where hi is bf16-representable and
    x == hi + lo exactly in fp32; bf16(lo) then loses only ~2^-17."""
    xb = np.ascontiguousarray(x.astype(np.float32))
    hi_bits = (xb.view(np.uint32) + 0x8000) & 0xFFFF0000
    hi = hi_bits.view(np.float32)
    lo = xb - hi
    return hi, lo


def _host_inputs(weights, positions):
    """Builds the per-core in_maps (host-side packing only, no math on data)."""
    import ml_dtypes

    w = np.ascontiguousarray(np.asarray(weights, dtype=np.float32))
    x = np.ascontiguousarray(np.asarray(positions, dtype=np.float32))

    # block-diagonal stationary operand, prescaled by 2^-14 (the A operand is
    # scaled by 2^14 on device so its fp16 cast stays in the normal range)
    # lhsT[k=(gg*16+b'), g, j, m=(d*16+b)] = delta(b,b') * w[16g+b, d, 8j+gg]
    w4 = w.reshape(NG, GB, D, NCHUNK, PG)  # [g, b, d, j, gg]
    eye = np.eye(GB, dtype=np.float32)
    lhsT = np.einsum("gbdjh,cb->hcgjdb", w4, eye)  # [gg, b', g, j, d, b]
    lhsT = np.ascontiguousarray(lhsT.reshape(128, NG, NCHUNK, 128)) * np.float32(
        2.0**-14
    )

    # broadcast selectors: sel_g(b2, m) = 1 iff b2 == 16g + m%16
    b2 = np.arange(32)[:, None]
    sel = np.zeros((32, NG, 128), dtype=np.float32)
    for g in range(NG):
        sel[:, g, :] = (b2 == (GB * g + np.arange(128)[None, :] % GB))
    lbc64 = np.ascontiguousarray(
        np.concatenate([sel, sel], axis=0).reshape(2, 32, NG, 128).repeat(2, 0)
    )
    # lbc64 rows: [0:32]=sel (hi), [32:64]=sel (lo), then repeated for window 1
    lbc64 = np.tile(np.concatenate([sel, sel], axis=0), (2, 1, 1)).astype(
        ml_dtypes.bfloat16
    )
    lbc16 = np.tile(sel, (4, 1, 1)).astype(np.float16)

    # per-partition activation scale/bias, partition p -> degree g = 8j + p//16
    gidx = np.arange(128) // GB
    scbi = np.zeros((128, 2 * NCHUNK), dtype=np.float32)
    for j in range(NCHUNK):
        c = _centers[PG * j + gidx]
        scbi[:, j] = (2.0 * INV2S) * c              # 25 c
        scbi[:, NCHUNK + j] = -INV2S * c * c        # -12.5 c^2

    in_maps = []
    for ci in range(NCORES):
        xs = x[:, ci * NSH : (ci + 1) * NSH]  # [32, NSH]
        # q-major packing: row q*32+b holds xs[b, 2048q:2048(q+1)]
        xpk = np.ascontiguousarray(
            xs.reshape(B, NQ, T2).transpose(1, 0, 2).reshape(128, T2)
        )
        hi, lo = _split_hi_lo(xpk)
        hib = hi.astype(ml_dtypes.bfloat16)
        lob = lo.astype(ml_dtypes.bfloat16)
        # K=64 windows: rows 64*qq+[0:32] = hi of q-block, +[32:64] = lo
        xhl = np.empty((2, 128, T2), dtype=ml_dtypes.bfloat16)
        for q in range(NQ):
            t, qq = divmod(q, 2)
            xhl[t, 64 * qq : 64 * qq + 32] = hib[32 * q : 32 * (q + 1)]
            xhl[t, 64 * qq + 32 : 64 * (qq + 1)] = lob[32 * q : 32 * (q + 1)]
        in_maps.append(
            {
                "xpk": xpk,
                "xhl0": np.ascontiguousarray(xhl[0]),
                "xhl1": np.ascontiguousarray(xhl[1]),
                "lhsTw": lhsT,
                "lbc64": lbc64,
                "lbc16": lbc16,
                "scbi": scbi,
            }
        )
    return in_maps


def _gather(results):
    """[NG, 128, NSH] per core, rows m=d*16+b  ->  full [B, N, D]."""
    outs = []
    for r in results:
        o = r["out"].reshape(NG, D, GB, NSH)  # [g, d, b, n]
        outs.append(o.transpose(0, 2, 3, 1).reshape(B, NSH, D))  # [b, n, d]
    full = np.concatenate(outs, axis=1)  # [B, N, D]
    return np.ascontiguousarray(full)


_NC_CACHE = {}


def run(inputs, trace=False, **trace_kwargs):
    """Builds (cached), runs on 8 cores, returns ((result, zeros), BassKernelResults)."""
    key = (F32R,)
    if key not in _NC_CACHE:
        _NC_CACHE[key] = _build()
    nc = _NC_CACHE[key]
    in_maps = _host_inputs(inputs["weights"], inputs["positions"])
    br = run_bass_kernel_spmd(
        nc, in_maps, list(range(NCORES)), trace=trace, **trace_kwargs
    )
    result = _gather(br.results)
    return (result, np.zeros_like(result)), br


def kernel(weights, weights_std, positions):
    out, _ = run(
        {"weights": weights, "weights_std": weights_std, "positions": positions}
    )
    return out
